# revision 10
# baseline (speedup 1.0000x reference)
"""Banded (context_window=256) multi-head attention for TRN2, 8 NeuronCores.

Sharding: core = (batch b, head-group hg of 4 heads); B=2 x 4 groups = 8 cores.
Each core computes Q/K/V projections for its 4 heads (tensor-parallel columns),
banded attention, and a row-parallel partial of the output projection
(attn_out_c @ Wo[rows_c]). Host sums the 4 partials per batch and adds the
exact bias correction (bv @ Wo + bo) - softmax rows sum to 1, so the V-bias
contributes a constant row vector.

Device layouts (all matmul operands float32r = fp32 storage, FP22 multiply):
  x^T   [1024, 2048]  host-pretransposed; contraction operand for all proj.
  Q^T/K^T [256, 2048] as 2 chunks [128, 2048] (2 heads per chunk), scaled 1/8.
  V     [2048, 260]   natural, 65-wide groups per head: [V_h (64) | ones].
  Scores per key-tile as S^T [128 keys, 384 queries] parallelogram (query
  window starts at the key-tile diagonal); one shared 0/1 band mask; no
  max-subtraction (scores are O(1), exp cannot overflow). Head pairs are
  interleaved so the two K=64 score matmuls run on disjoint PE row groups.
  exp -> mask-mul -> P^T tiles; AV accumulates [65, 512] PSUM per q-block via
  per-element has_written semantics (start=True only on the first, bank-clear);
  row 64 = softmax denominator l (ones column). 1/l = exp(-ln(l)) on ScalarE,
  broadcast across partitions by a K=1 outer-product matmul, normalization
  fused into a VectorE multiply.
"""

import json

import numpy as np

import concourse.bass as bass
import concourse.mybir as mybir
from concourse.tile import TileContext
from concourse.bass_utils import run_bass_kernel_spmd

F32 = mybir.dt.float32
F32R = mybir.dt.float32r
AFT = mybir.ActivationFunctionType

S = 2048          # sequence length
D = 1024          # model dim
HPC = 4           # heads per core
DH = 64           # head dim
DV = HPC * 65     # V width incl. ones columns = 260
W = 256           # context window
NKT = S // 128    # 16 key tiles
NQB = S // 512    # 4 q-blocks


# ---------------------------------------------------------------------------
# This toolchain's walrus accepts at most 1 sync-wait per regular instruction
# (2 per EventSemaphore), but Tile's wait assigner can emit more. Post-process
# the BIR: split excess waits onto NoOps injected just before the instruction.
_orig_to_json_bytes = bass.Bass.to_json_bytes


def _fix_module(m):
    for fn in m.get("functions", []):
        for blk in fn.get("blocks", []):
            insts = blk.get("instructions")
            if not insts:
                continue
            out = []
            for inst in insts:
                si = inst.get("sync_info") or {}
                ow = si.get("on_wait") or []
                lim = 2 if "EventSemaphore" in str(inst.get("opcode")) else 1
                if len(ow) > lim:
                    extra, keep = ow[:-lim], ow[-lim:]
                    for i, w in enumerate(extra):
                        out.append({
                            "debug": inst.get("debug", 0),
                            "engine": inst["engine"],
                            "ins": [],
                            "name": f"{inst['name']}ws{i}",
                            "opcode": "NoOp",
                            "outs": [],
                            "sync_info": {"on_update": [], "on_wait": [w]},
                        })
                    si["on_wait"] = keep
                out.append(inst)
            blk["instructions"] = out


def _to_json_bytes_fixed(self, *a, **kw):
    m = json.loads(_orig_to_json_bytes(self, *a, **kw))
    _fix_module(m)
    return json.dumps(m).encode()


bass.Bass.to_json_bytes = _to_json_bytes_fixed

import concourse.bass_utils as _bu
_orig_walrus_args = _bu.get_walrus_args


def _walrus_args_ldwopt(*a, **kw):
    return [x.replace("--enable-ldw-opt=false", "--enable-ldw-opt=true")
            for x in _orig_walrus_args(*a, **kw)]


_bu.get_walrus_args = _walrus_args_ldwopt
# ---------------------------------------------------------------------------


def build_nc():
    nc = bass.Bass()
    xT = nc.dram_tensor("xt", [D, S], F32R, kind="ExternalInput")
    wq = nc.dram_tensor("wq", [D, 256], F32R, kind="ExternalInput")
    wk = nc.dram_tensor("wk", [D, 256], F32R, kind="ExternalInput")
    wv = nc.dram_tensor("wv", [D, DV], F32R, kind="ExternalInput")
    wo = nc.dram_tensor("wo", [256, D], F32R, kind="ExternalInput")
    bias = nc.dram_tensor("bias", [128, 4], F32, kind="ExternalInput")
    mask = nc.dram_tensor("mask", [128, 384], F32R, kind="ExternalInput")
    out = nc.dram_tensor("out", [S, D], F32, kind="ExternalOutput")

    with TileContext(nc) as tc, \
         nc.allow_low_precision(reason="fp32r storage, fp32 PSUM accumulate"):
        with tc.tile_pool(name="const", bufs=1) as cpool, \
             tc.tile_pool(name="qkv", bufs=1) as qkvpool, \
             tc.tile_pool(name="attn", bufs=1) as apool:
            # ---- constants ----
            wq_sb = [cpool.tile([128, 256], F32R, tag=f"wq{d}", name=f"wq{d}")
                     for d in range(8)]
            wk_sb = [cpool.tile([128, 256], F32R, tag=f"wk{d}", name=f"wk{d}")
                     for d in range(8)]
            wv_sb = [cpool.tile([128, DV], F32R, tag=f"wv{d}", name=f"wv{d}")
                     for d in range(8)]
            wo_sb = [cpool.tile([128, D], F32R, tag=f"wo{c}", name=f"wo{c}")
                     for c in range(2)]
            mask_sb = cpool.tile([128, 384], F32R, tag="mask")
            bias_sb = cpool.tile([128, 4], F32, tag="bias")
            ones_sb = cpool.tile([65, 64], F32R, tag="ones")
            nc.sync.dma_start(bias_sb[:], bias[:])
            nc.sync.dma_start(mask_sb[:], mask[:])
            nc.vector.memset(ones_sb[:].bitcast(F32), 1.0)

            # ---- persistent activations ----
            qt_sb = [qkvpool.tile([128, S], F32R, tag=f"qt{c}", name=f"qt{c}")
                     for c in range(2)]
            kt_sb = [qkvpool.tile([128, S], F32R, tag=f"kt{c}", name=f"kt{c}")
                     for c in range(2)]
            v_sb = [qkvpool.tile([128, DV], F32R, tag=f"v{s}", name=f"v{s}")
                    for s in range(NKT)]
            attn_sb = [apool.tile([128, S], F32R, tag=f"at{c}", name=f"at{c}")
                       for c in range(2)]

            # ---- phase 1: projections (weight-stationary d-loops) ----
            with tc.tile_pool(name="xt", bufs=1) as xpool, \
                 tc.tile_pool(name="pps", bufs=6, space="PSUM") as pps:
                xt_sb = [xpool.tile([128, S], F32R, tag=f"x{d}", name=f"x{d}")
                         for d in range(8)]
                for d in range(8):
                    r = slice(128 * d, 128 * d + 128)
                    nc.sync.dma_start(wq_sb[d][:], wq[r, :])
                    nc.sync.dma_start(xt_sb[d][:], xT[r, :])
                    nc.sync.dma_start(wk_sb[d][:], wk[r, :])
                    nc.sync.dma_start(wv_sb[d][:], wv[r, :])
                for c in range(2):
                    nc.sync.dma_start(wo_sb[c][:],
                                      wo[128 * c:128 * c + 128, :])

                # Q^T and K^T: one weight load per (w, c, d); the 4 q-block
                # matmuls reuse the loaded weights.
                def proj_qk(wsb, dst, bcol, scale, c):
                    grp = [pps.tile([128, 512], F32, tag="p",
                                    name=f"pj{bcol}{c}{qb}")
                           for qb in range(NQB)]
                    for d in range(8):
                        for qb in range(NQB):
                            nc.tensor.matmul(
                                grp[qb][:],
                                wsb[d][:, 128 * c:128 * c + 128],
                                xt_sb[d][:, 512 * qb:512 * qb + 512],
                                start=(d == 0), stop=(d == 7))
                    for qb in range(NQB):
                        nc.scalar.activation(
                            dst[c][:, 512 * qb:512 * qb + 512], grp[qb][:],
                            AFT.Identity,
                            bias=bias_sb[:, bcol + c:bcol + c + 1],
                            scale=scale)

                def proj_v():
                    # lhsT changes every matmul (x^T slice), rhs stationary
                    for sc in range(NKT):
                        s0 = 128 * sc
                        ps = pps.tile([128, DV], F32, tag="p", name=f"pv{sc}")
                        for d in range(8):
                            nc.tensor.matmul(
                                ps[:], xt_sb[d][:, s0:s0 + 128], wv_sb[d][:],
                                start=(d == 0), stop=(d == 7))
                        nc.vector.tensor_copy(v_sb[sc][:], ps[:])
                        for h in range(HPC):
                            nc.vector.memset(
                                v_sb[sc][:, 65 * h + 64:65 * h + 65]
                                .bitcast(F32), 1.0)

                # pair-0 inputs first so attention starts early; c=1
                # projections then fill PE gaps during pair-0 attention.
                proj_qk(wq_sb, qt_sb, 0, 0.125, 0)
                proj_qk(wk_sb, kt_sb, 2, 1.0, 0)
                proj_v()
                proj_qk(wq_sb, qt_sb, 0, 0.125, 1)
                proj_qk(wk_sb, kt_sb, 2, 1.0, 1)

            # ---- phase 2: banded attention, head pairs interleaved ----
            with tc.tile_pool(name="p", bufs=36) as ppool, \
                 tc.tile_pool(name="tmp", bufs=2) as tpool, \
                 tc.tile_pool(name="sps", bufs=3, space="PSUM") as sps, \
                 tc.tile_pool(name="avps", bufs=3, space="PSUM") as avps, \
                 tc.tile_pool(name="bcps", bufs=2, space="PSUM") as bcps:
                for pair in range(2):
                    c = pair
                    p_tiles = {0: [], 1: []}  # per head-in-pair

                    def scores_kt(j, pair=None, c=None, p_tiles=None):
                        k0 = 128 * j
                        w = min(384, S - k0)
                        for hi in range(2):
                            r = hi * 64
                            ps = sps.tile([128, 384], F32, tag="s",
                                          name=f"s{pair}{j}{hi}")
                            nc.tensor.matmul(
                                ps[:, :w], kt_sb[c][r:r + 64, k0:k0 + 128],
                                qt_sb[c][r:r + 64, k0:k0 + w],
                                start=True, stop=True,
                                tile_position=(r, 0))
                            pt = ppool.tile([128, 384], F32R, tag="p",
                                            name=f"p{pair}{j}{hi}")
                            nc.scalar.activation(pt[:, :w], ps[:, :w], AFT.Exp)
                            eng = nc.vector if hi == 0 else nc.gpsimd
                            eng.tensor_mul(pt[:, :w], pt[:, :w],
                                           mask_sb[:, :w])
                            p_tiles[hi].append(pt)

                    # AV groups; bcast matmuls lag 2 groups so the ln/exp
                    # reciprocal latency hides under PE work.
                    pend = []

                    def flush(n, c=c, pend=pend):
                        while len(pend) > n:
                            av, hh, qq = pend.pop(0)
                            pr = c
                            lnl = tpool.tile([65, 512], F32, tag="lnl",
                                             name=f"ln{pr}{hh}{qq}")
                            nc.scalar.activation(lnl[64:65, :], av[64:65, :],
                                                 AFT.Ln)
                            recip = tpool.tile([65, 512], F32R, tag="recip",
                                               name=f"rc{pr}{hh}{qq}")
                            nc.scalar.activation(recip[64:65, :],
                                                 lnl[64:65, :], AFT.Exp,
                                                 scale=-1.0)
                            bc = bcps.tile([64, 512], F32, tag="bc",
                                           name=f"bc{pr}{hh}{qq}")
                            nc.tensor.matmul(bc[:], ones_sb[64:65, :],
                                             recip[64:65, :],
                                             start=True, stop=True)
                            bcs = tpool.tile([64, 512], F32, tag="bcs",
                                             name=f"bs{pr}{hh}{qq}")
                            nc.vector.tensor_copy(bcs[:], bc[:])
                            q0 = 512 * qq
                            if hh == 0:
                                dst = attn_sb[c][0:64, q0:q0 + 512]
                                nc.vector.tensor_mul(dst, av[0:64, :], bcs[:])
                            else:
                                tq = tpool.tile([64, 512], F32R, tag="tmp",
                                                name=f"tq{c}{qq}", bufs=3)
                                nc.vector.tensor_mul(tq[:], av[0:64, :],
                                                     bcs[:])
                                nc.sync.dma_start(
                                    attn_sb[c][64:128, q0:q0 + 512], tq[:])

                    for qb in range(NQB):
                        # emit this q-block's score tiles just before its AV;
                        # AV of block n overlaps the exp drain of block n+1
                        for j in range(4 * qb, min(4 * qb + 4, NKT)):
                            scores_kt(j, pair=pair, c=c, p_tiles=p_tiles)
                        q0 = 512 * qb
                        js = [j for j in range(4 * qb - 2, 4 * qb + 4)
                              if 0 <= j < NKT]
                        for hi in range(2):
                            h = 2 * pair + hi
                            av = avps.tile([65, 512], F32, tag="av",
                                           name=f"av{pair}{hi}{qb}")
                            for i, j in enumerate(js):
                                k0 = 128 * j
                                w = min(384, S - k0)
                                lo = max(q0, k0)
                                hi_ = min(q0 + 512, k0 + w)
                                nc.tensor.matmul(
                                    av[0:65, lo - q0:hi_ - q0],
                                    v_sb[j][:, 65 * h:65 * h + 65],
                                    p_tiles[hi][j][:, lo - k0:hi_ - k0],
                                    start=(i == 0), stop=(i == len(js) - 1))
                            pend.append((av, hi, qb))
                            flush(2)
                    flush(0)

            # ---- phase 3: output projection (row-parallel partial) ----
            with tc.tile_pool(name="ost", bufs=3) as opool, \
                 tc.tile_pool(name="fps", bufs=4, space="PSUM") as fps:
                for sc in range(NKT):
                    s0 = 128 * sc
                    o_sb = opool.tile([128, D], F32, tag="o", name=f"o{sc}")
                    for nh in range(2):
                        n0 = 512 * nh
                        ps = fps.tile([128, 512], F32, tag="f",
                                      name=f"f{sc}{nh}")
                        for c in range(2):
                            nc.tensor.matmul(
                                ps[:], attn_sb[c][:, s0:s0 + 128],
                                wo_sb[c][:, n0:n0 + 512],
                                start=(c == 0), stop=(c == 1))
                        if nh == 0:
                            nc.vector.tensor_copy(o_sb[:, n0:n0 + 512], ps[:])
                        else:
                            nc.scalar.activation(
                                o_sb[:, n0:n0 + 512], ps[:], AFT.Copy)
                    nc.sync.dma_start(out[s0:s0 + 128, :], o_sb[:])
    return nc


_NC = None
_last_in_maps = None


def _get_nc():
    global _NC
    if _NC is None:
        _NC = build_nc()
    return _NC


def _band_mask_tile():
    ki = np.arange(128)[:, None]
    qo = np.arange(384)[None, :]
    d = qo - ki
    return ((d >= 0) & (d < W)).astype(np.float32)


def kernel(query, Wq, bq, Wk, bk, Wv, bv, Wo, bo):
    query = np.asarray(query, np.float32)
    Wq, bq = np.asarray(Wq, np.float32), np.asarray(bq, np.float32)
    Wk, bk = np.asarray(Wk, np.float32), np.asarray(bk, np.float32)
    Wv, bv = np.asarray(Wv, np.float32), np.asarray(bv, np.float32)
    Wo, bo = np.asarray(Wo, np.float32), np.asarray(bo, np.float32)
    B = query.shape[0]
    mask = _band_mask_tile()

    in_maps = []
    for b in range(B):
        xT = np.ascontiguousarray(query[b].T)
        for hg in range(4):
            cols = slice(hg * 256, hg * 256 + 256)
            wv_c = np.zeros((D, DV), np.float32)
            for h in range(HPC):
                src = hg * 256 + DH * h
                wv_c[:, 65 * h:65 * h + DH] = Wv[:, src:src + DH]
            bias_c = np.zeros((128, 4), np.float32)
            bias_c[:, 0] = bq[cols][0:128] * 0.125
            bias_c[:, 1] = bq[cols][128:256] * 0.125
            bias_c[:, 2] = bk[cols][0:128]
            bias_c[:, 3] = bk[cols][128:256]
            in_maps.append({
                "xt": xT,
                "wq": np.ascontiguousarray(Wq[:, cols]),
                "wk": np.ascontiguousarray(Wk[:, cols]),
                "wv": wv_c,
                "wo": np.ascontiguousarray(Wo[cols, :]),
                "bias": bias_c,
                "mask": mask,
            })

    global _last_in_maps
    _last_in_maps = in_maps
    res = run_bass_kernel_spmd(_get_nc(), in_maps, core_ids=list(range(8)))
    corr = (bv @ Wo + bo).astype(np.float32)
    out = np.empty((B, S, D), np.float32)
    for b in range(B):
        acc = res.results[4 * b]["out"].copy()
        for hg in range(1, 4):
            acc += res.results[4 * b + hg]["out"]
        out[b] = acc + corr
    return out


# revision 11
# speedup vs baseline: 1.1300x; 1.1300x over previous
"""Banded (context_window=256) multi-head attention for TRN2, 8 NeuronCores.

Sharding: core = (batch b, head-group hg of 4 heads); B=2 x 4 groups = 8 cores.
Each core computes Q/K/V projections for its 4 heads (tensor-parallel columns),
banded attention, and a row-parallel partial of the output projection
(attn_out_c @ Wo[rows_c]). Host sums the 4 partials per batch and adds the
exact bias correction (bv @ Wo + bo) - softmax rows sum to 1, so the V-bias
contributes a constant row vector.

Device layouts (all matmul operands float32r = fp32 storage, FP22 multiply):
  x^T   [1024, 2048]  host-pretransposed; contraction operand for all proj.
  Q^T/K^T [256, 2048] as 2 chunks [128, 2048] (2 heads per chunk), scaled 1/8.
  V     [2048, 260]   natural, 65-wide groups per head: [V_h (64) | ones].
  Scores per key-tile as S^T [128 keys, 384 queries] parallelogram (query
  window starts at the key-tile diagonal); one shared 0/1 band mask; no
  max-subtraction (scores are O(1), exp cannot overflow). Head pairs are
  interleaved so the two K=64 score matmuls run on disjoint PE row groups.
  exp -> mask-mul -> P^T tiles; AV accumulates [65, 512] PSUM per q-block via
  per-element has_written semantics (start=True only on the first, bank-clear);
  row 64 = softmax denominator l (ones column). 1/l = exp(-ln(l)) on ScalarE,
  broadcast across partitions by a K=1 outer-product matmul, normalization
  fused into a VectorE multiply.
"""

import json

import numpy as np

import concourse.bass as bass
import concourse.mybir as mybir
from concourse.tile import TileContext
from concourse.bass_utils import run_bass_kernel_spmd

F32 = mybir.dt.float32
F32R = mybir.dt.float32r
AFT = mybir.ActivationFunctionType

S = 2048          # sequence length
D = 1024          # model dim
HPC = 4           # heads per core
DH = 64           # head dim
DV = HPC * 65     # V width incl. ones columns = 260
W = 256           # context window
NKT = S // 128    # 16 key tiles
NQB = S // 512    # 4 q-blocks


# ---------------------------------------------------------------------------
# This toolchain's walrus accepts at most 1 sync-wait per regular instruction
# (2 per EventSemaphore), but Tile's wait assigner can emit more. Post-process
# the BIR: split excess waits onto NoOps injected just before the instruction.
_orig_to_json_bytes = bass.Bass.to_json_bytes


def _fix_module(m):
    for fn in m.get("functions", []):
        for blk in fn.get("blocks", []):
            insts = blk.get("instructions")
            if not insts:
                continue
            out = []
            for inst in insts:
                si = inst.get("sync_info") or {}
                ow = si.get("on_wait") or []
                lim = 2 if "EventSemaphore" in str(inst.get("opcode")) else 1
                if len(ow) > lim:
                    extra, keep = ow[:-lim], ow[-lim:]
                    for i, w in enumerate(extra):
                        out.append({
                            "debug": inst.get("debug", 0),
                            "engine": inst["engine"],
                            "ins": [],
                            "name": f"{inst['name']}ws{i}",
                            "opcode": "NoOp",
                            "outs": [],
                            "sync_info": {"on_update": [], "on_wait": [w]},
                        })
                    si["on_wait"] = keep
                out.append(inst)
            blk["instructions"] = out


def _to_json_bytes_fixed(self, *a, **kw):
    m = json.loads(_orig_to_json_bytes(self, *a, **kw))
    _fix_module(m)
    return json.dumps(m).encode()


bass.Bass.to_json_bytes = _to_json_bytes_fixed

import concourse.bass_utils as _bu
_orig_walrus_args = _bu.get_walrus_args


def _walrus_args_ldwopt(*a, **kw):
    return [x.replace("--enable-ldw-opt=false", "--enable-ldw-opt=true")
            for x in _orig_walrus_args(*a, **kw)]


_bu.get_walrus_args = _walrus_args_ldwopt
# ---------------------------------------------------------------------------


def build_nc():
    nc = bass.Bass()
    xT = nc.dram_tensor("xt", [D, S], F32R, kind="ExternalInput")
    wq = nc.dram_tensor("wq", [D, 256], F32R, kind="ExternalInput")
    wk = nc.dram_tensor("wk", [D, 256], F32R, kind="ExternalInput")
    wv = nc.dram_tensor("wv", [D, DV], F32R, kind="ExternalInput")
    wo = nc.dram_tensor("wo", [256, D], F32R, kind="ExternalInput")
    bias = nc.dram_tensor("bias", [128, 4], F32, kind="ExternalInput")
    mask = nc.dram_tensor("mask", [128, 384], F32R, kind="ExternalInput")
    out = nc.dram_tensor("out", [S, D], F32, kind="ExternalOutput")

    with TileContext(nc) as tc, \
         nc.allow_low_precision(reason="fp32r storage, fp32 PSUM accumulate"):
        with tc.tile_pool(name="const", bufs=1) as cpool, \
             tc.tile_pool(name="qkv", bufs=1) as qkvpool, \
             tc.tile_pool(name="attn", bufs=1) as apool:
            # ---- constants ----
            wq_sb = [cpool.tile([128, 256], F32R, tag=f"wq{d}", name=f"wq{d}")
                     for d in range(8)]
            wk_sb = [cpool.tile([128, 256], F32R, tag=f"wk{d}", name=f"wk{d}")
                     for d in range(8)]
            wv_sb = [cpool.tile([128, DV], F32R, tag=f"wv{d}", name=f"wv{d}")
                     for d in range(8)]
            wo_sb = [cpool.tile([128, D], F32R, tag=f"wo{c}", name=f"wo{c}")
                     for c in range(2)]
            mask_sb = cpool.tile([128, 384], F32R, tag="mask")
            bias_sb = cpool.tile([128, 4], F32, tag="bias")
            ones_sb = cpool.tile([65, 64], F32R, tag="ones")
            nc.sync.dma_start(bias_sb[:], bias[:])
            nc.sync.dma_start(mask_sb[:], mask[:])
            nc.vector.memset(ones_sb[:].bitcast(F32), 1.0)

            # ---- persistent activations ----
            qt_sb = [qkvpool.tile([128, S], F32R, tag=f"qt{c}", name=f"qt{c}")
                     for c in range(2)]
            kt_sb = [qkvpool.tile([128, S], F32R, tag=f"kt{c}", name=f"kt{c}")
                     for c in range(2)]
            v_sb = [qkvpool.tile([128, DV], F32R, tag=f"v{s}", name=f"v{s}")
                    for s in range(NKT)]
            attn_sb = [apool.tile([128, S], F32R, tag=f"at{c}", name=f"at{c}")
                       for c in range(2)]

            # ---- phase 1: projections (weight-stationary d-loops) ----
            with tc.tile_pool(name="xt", bufs=1) as xpool, \
                 tc.tile_pool(name="pps", bufs=6, space="PSUM") as pps:
                xt_sb = [xpool.tile([128, S], F32R, tag=f"x{d}", name=f"x{d}")
                         for d in range(8)]
                for d in range(8):
                    r = slice(128 * d, 128 * d + 128)
                    nc.sync.dma_start(wq_sb[d][:], wq[r, :])
                    nc.sync.dma_start(xt_sb[d][:], xT[r, :])
                    nc.sync.dma_start(wk_sb[d][:], wk[r, :])
                    nc.sync.dma_start(wv_sb[d][:], wv[r, :])
                for c in range(2):
                    nc.sync.dma_start(wo_sb[c][:],
                                      wo[128 * c:128 * c + 128, :])

                # Q^T and K^T: one weight load per (w, c, d); the 4 q-block
                # matmuls reuse the loaded weights.
                def proj_qk(wsb, dst, bcol, scale, c):
                    grp = [pps.tile([128, 512], F32, tag="p",
                                    name=f"pj{bcol}{c}{qb}")
                           for qb in range(NQB)]
                    for d in range(8):
                        for qb in range(NQB):
                            nc.tensor.matmul(
                                grp[qb][:],
                                wsb[d][:, 128 * c:128 * c + 128],
                                xt_sb[d][:, 512 * qb:512 * qb + 512],
                                start=(d == 0), stop=(d == 7))
                    for qb in range(NQB):
                        nc.scalar.activation(
                            dst[c][:, 512 * qb:512 * qb + 512], grp[qb][:],
                            AFT.Identity,
                            bias=bias_sb[:, bcol + c:bcol + c + 1],
                            scale=scale)

                def proj_v():
                    # lhsT changes every matmul (x^T slice), rhs stationary
                    for sc in range(NKT):
                        s0 = 128 * sc
                        ps = pps.tile([128, DV], F32, tag="p", name=f"pv{sc}")
                        for d in range(8):
                            nc.tensor.matmul(
                                ps[:], xt_sb[d][:, s0:s0 + 128], wv_sb[d][:],
                                start=(d == 0), stop=(d == 7))
                        nc.vector.tensor_copy(v_sb[sc][:], ps[:])
                        for h in range(HPC):
                            nc.vector.memset(
                                v_sb[sc][:, 65 * h + 64:65 * h + 65]
                                .bitcast(F32), 1.0)

                # pair-0 inputs first so attention starts early; c=1
                # projections then fill PE gaps during pair-0 attention.
                proj_qk(wq_sb, qt_sb, 0, 0.125, 0)
                proj_qk(wk_sb, kt_sb, 2, 1.0, 0)
                proj_v()
                proj_qk(wq_sb, qt_sb, 0, 0.125, 1)
                proj_qk(wk_sb, kt_sb, 2, 1.0, 1)

            # ---- phase 2: banded attention, head pairs interleaved ----
            with tc.tile_pool(name="p", bufs=36) as ppool, \
                 tc.tile_pool(name="tmp", bufs=2) as tpool, \
                 tc.tile_pool(name="sps", bufs=3, space="PSUM") as sps, \
                 tc.tile_pool(name="avps", bufs=3, space="PSUM") as avps, \
                 tc.tile_pool(name="bcps", bufs=2, space="PSUM") as bcps:
                for pair in range(2):
                    c = pair
                    p_tiles = {0: [], 1: []}  # per head-in-pair

                    def scores_kt(j, pair=None, c=None, p_tiles=None):
                        k0 = 128 * j
                        w = min(384, S - k0)
                        for hi in range(2):
                            r = hi * 64
                            ps = sps.tile([128, 384], F32, tag="s",
                                          name=f"s{pair}{j}{hi}")
                            nc.tensor.matmul(
                                ps[:, :w], kt_sb[c][r:r + 64, k0:k0 + 128],
                                qt_sb[c][r:r + 64, k0:k0 + w],
                                start=True, stop=True,
                                tile_position=(r, 0))
                            pt = ppool.tile([128, 384], F32R, tag="p",
                                            name=f"p{pair}{j}{hi}")
                            nc.scalar.activation(pt[:, :w], ps[:, :w], AFT.Exp)
                            eng = nc.vector if hi == 0 else nc.gpsimd
                            eng.tensor_mul(pt[:, :w], pt[:, :w],
                                           mask_sb[:, :w])
                            p_tiles[hi].append(pt)

                    # AV groups; bcast matmuls lag 2 groups so the ln/exp
                    # reciprocal latency hides under PE work.
                    pend = []

                    def flush(n, c=c, pend=pend):
                        while len(pend) > n:
                            av, hh, qq = pend.pop(0)
                            pr = c
                            lnl = tpool.tile([65, 512], F32, tag="lnl",
                                             name=f"ln{pr}{hh}{qq}")
                            nc.scalar.activation(lnl[64:65, :], av[64:65, :],
                                                 AFT.Ln)
                            recip = tpool.tile([65, 512], F32R, tag="recip",
                                               name=f"rc{pr}{hh}{qq}")
                            nc.scalar.activation(recip[64:65, :],
                                                 lnl[64:65, :], AFT.Exp,
                                                 scale=-1.0)
                            bc = bcps.tile([64, 512], F32, tag="bc",
                                           name=f"bc{pr}{hh}{qq}")
                            nc.tensor.matmul(bc[:], ones_sb[64:65, :],
                                             recip[64:65, :],
                                             start=True, stop=True)
                            bcs = tpool.tile([64, 512], F32, tag="bcs",
                                             name=f"bs{pr}{hh}{qq}")
                            nc.vector.tensor_copy(bcs[:], bc[:])
                            q0 = 512 * qq
                            if hh == 0:
                                dst = attn_sb[c][0:64, q0:q0 + 512]
                                nc.vector.tensor_mul(dst, av[0:64, :], bcs[:])
                            else:
                                tq = tpool.tile([64, 512], F32R, tag="tmp",
                                                name=f"tq{c}{qq}", bufs=3)
                                nc.vector.tensor_mul(tq[:], av[0:64, :],
                                                     bcs[:])
                                nc.sync.dma_start(
                                    attn_sb[c][64:128, q0:q0 + 512], tq[:])

                    kt_done = 0
                    for qb in range(NQB):
                        # scores run one q-block ahead of AV so the exp/mask
                        # drain of block n+1 overlaps block n's AV matmuls
                        for j in range(kt_done, min(4 * qb + 8, NKT)):
                            scores_kt(j, pair=pair, c=c, p_tiles=p_tiles)
                            kt_done = j + 1
                        q0 = 512 * qb
                        js = [j for j in range(4 * qb - 2, 4 * qb + 4)
                              if 0 <= j < NKT]
                        for hi in range(2):
                            h = 2 * pair + hi
                            av = avps.tile([65, 512], F32, tag="av",
                                           name=f"av{pair}{hi}{qb}")
                            for i, j in enumerate(js):
                                k0 = 128 * j
                                w = min(384, S - k0)
                                lo = max(q0, k0)
                                hi_ = min(q0 + 512, k0 + w)
                                nc.tensor.matmul(
                                    av[0:65, lo - q0:hi_ - q0],
                                    v_sb[j][:, 65 * h:65 * h + 65],
                                    p_tiles[hi][j][:, lo - k0:hi_ - k0],
                                    start=(i == 0), stop=(i == len(js) - 1))
                            pend.append((av, hi, qb))
                            flush(2)
                    flush(0)

            # ---- phase 3: output projection (row-parallel partial) ----
            with tc.tile_pool(name="ost", bufs=3) as opool, \
                 tc.tile_pool(name="fps", bufs=4, space="PSUM") as fps:
                for sc in range(NKT):
                    s0 = 128 * sc
                    o_sb = opool.tile([128, D], F32, tag="o", name=f"o{sc}")
                    for nh in range(2):
                        n0 = 512 * nh
                        ps = fps.tile([128, 512], F32, tag="f",
                                      name=f"f{sc}{nh}")
                        for c in range(2):
                            nc.tensor.matmul(
                                ps[:], attn_sb[c][:, s0:s0 + 128],
                                wo_sb[c][:, n0:n0 + 512],
                                start=(c == 0), stop=(c == 1))
                        if nh == 0:
                            nc.vector.tensor_copy(o_sb[:, n0:n0 + 512], ps[:])
                        else:
                            nc.scalar.activation(
                                o_sb[:, n0:n0 + 512], ps[:], AFT.Copy)
                    nc.sync.dma_start(out[s0:s0 + 128, :], o_sb[:])
    return nc


_NC = None
_last_in_maps = None


def _get_nc():
    global _NC
    if _NC is None:
        _NC = build_nc()
    return _NC


def _band_mask_tile():
    ki = np.arange(128)[:, None]
    qo = np.arange(384)[None, :]
    d = qo - ki
    return ((d >= 0) & (d < W)).astype(np.float32)


def kernel(query, Wq, bq, Wk, bk, Wv, bv, Wo, bo):
    query = np.asarray(query, np.float32)
    Wq, bq = np.asarray(Wq, np.float32), np.asarray(bq, np.float32)
    Wk, bk = np.asarray(Wk, np.float32), np.asarray(bk, np.float32)
    Wv, bv = np.asarray(Wv, np.float32), np.asarray(bv, np.float32)
    Wo, bo = np.asarray(Wo, np.float32), np.asarray(bo, np.float32)
    B = query.shape[0]
    mask = _band_mask_tile()

    in_maps = []
    for b in range(B):
        xT = np.ascontiguousarray(query[b].T)
        for hg in range(4):
            cols = slice(hg * 256, hg * 256 + 256)
            wv_c = np.zeros((D, DV), np.float32)
            for h in range(HPC):
                src = hg * 256 + DH * h
                wv_c[:, 65 * h:65 * h + DH] = Wv[:, src:src + DH]
            bias_c = np.zeros((128, 4), np.float32)
            bias_c[:, 0] = bq[cols][0:128] * 0.125
            bias_c[:, 1] = bq[cols][128:256] * 0.125
            bias_c[:, 2] = bk[cols][0:128]
            bias_c[:, 3] = bk[cols][128:256]
            in_maps.append({
                "xt": xT,
                "wq": np.ascontiguousarray(Wq[:, cols]),
                "wk": np.ascontiguousarray(Wk[:, cols]),
                "wv": wv_c,
                "wo": np.ascontiguousarray(Wo[cols, :]),
                "bias": bias_c,
                "mask": mask,
            })

    global _last_in_maps
    _last_in_maps = in_maps
    res = run_bass_kernel_spmd(_get_nc(), in_maps, core_ids=list(range(8)))
    corr = (bv @ Wo + bo).astype(np.float32)
    out = np.empty((B, S, D), np.float32)
    for b in range(B):
        acc = res.results[4 * b]["out"].copy()
        for hg in range(1, 4):
            acc += res.results[4 * b + hg]["out"]
        out[b] = acc + corr
    return out
